# revision 25
# baseline (speedup 1.0000x reference)
"""Trainium2 Bass kernel for nn_Attention_58815282151556 (sparse_attention).

Reference computation (per batch b):
    h_att  = h_prev @ W_h.T + b_h                       # [B, ATT]
    act    = relu(h_att[:, None, :] + features_proj)    # [B, L, ATT]
    scores = einsum("bla,a->bl", act, w_out) + b_out    # [B, L]
    alpha  = softmax(scores, axis=1)                    # [B, L]
    out    = einsum("bl,bld->bd", alpha, features)      # [B, ATT]

b_out is a constant shift on scores -> softmax-invariant -> dropped exactly.
Sharding: data-parallel over batch, 8 cores x 128 batches, weights
replicated, no cross-core communication.

Host-side preconditioning (exact algebra + fp16 casts + layout only):
  * w is folded into the score path: the kernel streams
    fpw = (features_proj * w)[..., perm] and projects with
    Wt = ((w[:,None]*W_h)[perm]).T, bw = (w*b_h)[perm], where perm sorts
    the attention axis so all w>=0 slots come first (P of them).  With
    zt = w*z = hattw + fpw:
        w_a*relu(z_a) = max(zt_a,0) if w_a>=0 else min(zt_a,0)
    so scores[b,l] is a plain unsigned sum once the clamp direction is
    split at column P.  The permutation only reorders the summed axis.
  * features are host-packed into G[q][p][s][a]: 4-batch groups q, step
    s covering an l-range of 24/25, p = j*ns+li stacking the 4 batches'
    l-slices in the partition dim.  Tiles are [<=100, 16KB rows] -- the
    fast DMA pattern (~900 GB/s/core measured vs ~220 GB/s for
    interleaved 2KB chunks).

Per-core design (103 MB fp16 streams):
  Phase A (scores): fpw streams as [128, 8*1024] fp16 chunks (2 MB,
    16KB rows).  DVE adds hattw broadcast over l (tensor_tensor, 2x),
    the clamp splits between DVE tensor_scalar (max, 4x) on the w>=0
    block and ACT Relu(-x) on the w<0 block, the fold 1024->128 runs on
    the otherwise-idle PE (identity matmuls accumulating 8 a-blocks in
    PSUM), and a DVE tensor_reduce tail emits fp32 scores per l.
  Softmax: no max-subtraction (scores are O(10); fp32 exp is safe);
    exp with accum -> Z, reciprocal, normalize to alpha fp16; PE
    transposes alpha; DVE builds the block-diagonal step matrices.
  Phase B (context): per 4-batch group one [<=100, 8*1024] fp16 tile;
    16 matmuls against block-diag alpha columns accumulate ctx[4,1024]
    in PSUM (K<=100, 8 l-steps x 2 psum halves); 4 PSUM tiles rotate;
    ACT drains [4,1024] tiles (32 ops total) into staging rows DMA'd
    out 4 batches at a time.

Cost-model engine budget: DVE ~160us, ACT ~125us, PE ~195us total
(fold 85 + context 109), DMA ~115us at the measured fast-pattern rate.
"""

import sys

for _p in ("/opt/trn_rl_repo",):
    if _p not in sys.path:
        sys.path.insert(0, _p)

import numpy as np

import concourse.bacc as bacc
import concourse.bass as bass
import concourse.tile as tile
from concourse import mybir
from concourse.masks import make_identity

B, L, ATT, HID = 1024, 196, 1024, 1024
NCORES = 8
BS = B // NCORES  # batches per core
L2 = L // 2  # 98
HALF = ATT // 2  # 512

F32 = mybir.dt.float32
F16 = mybir.dt.float16
OP = mybir.AluOpType
AF = mybir.ActivationFunctionType
AX = mybir.AxisListType

# l-chunking for phase A: 24 chunks of 8 plus one of 4
CHUNKS = [(c * 8, 8) for c in range(24)] + [(192, 4)]

# phase-B l-steps within each half: offsets/sizes covering 98
STEPS_HALF = [(0, 25), (25, 25), (50, 24), (74, 24)]
# (half, l-offset, ns) for all 8 steps
STEPS = [(h, off, ns) for h in (0, 1) for off, ns in STEPS_HALF]
NQ = BS // 4  # 32 four-batch groups
# batch j's rows sit at partition base 32*j (engine partition windows
# must start on 32-aligned bases); rows [32j+ns, 32j+32) are padding.
GP = 128


def _emit(tc, outs, ins, P):
    nc = tc.nc
    fpw_d = ins["fpw"]  # [BS, L, ATT] fp16
    G_d = ins["G"]  # [NQ, GP, 8, ATT] fp16
    h_d = ins["h"]  # [BS, HID] fp16
    Wt_d = ins["Wt"]  # [HID, ATT] fp16
    bw_d = ins["bw"]  # [ATT] fp32
    ctx_d = outs["ctx"]  # [BS, ATT] fp32

    KH = HID // 128
    dbg = {k: outs[k] for k in ("hattw", "scores", "alpha") if k in outs}

    import contextlib

    with contextlib.ExitStack() as es:
        consts = es.enter_context(tc.tile_pool(name="consts", bufs=1))
        ident = consts.tile([128, 128], F16)
        make_identity(nc, ident)
        hattw = consts.tile([128, ATT], F16)
        scores = consts.tile([128, L], F32)
        alpha = consts.tile([128, L], F16)
        # per-step transposed alpha slices (each at partition base 0)
        aTs = [
            consts.tile([25, 128], F16, name=f"aTs{s}") for s in range(8)
        ]
        # block-diag step matrices: col = s*128 + q*4 + j
        T = consts.tile([GP, 8 * 128], F16)
        nc.gpsimd.memset(T, 0.0)
        fb_pool = es.enter_context(tc.tile_pool(name="fb", bufs=4))
        stage_pool = es.enter_context(tc.tile_pool(name="stg", bufs=2))

        # ------------- setup: hattw = h @ Wt + bw ------------------------
        with tc.tile_pool(name="setup", bufs=1, side="right") as setup, \
                tc.tile_pool(name="setup2", bufs=2, side="right") as setup2, \
                tc.tile_pool(name="setup_ps", bufs=2, space="PSUM") as sps, \
                tc.tile_pool(name="hatt_ps", bufs=1, space="PSUM") as hps_p:
            h_sb = setup.tile([128, HID], F16)
            nc.sync.dma_start(out=h_sb, in_=h_d)
            bw_sb = setup.tile([1, ATT], F32)
            nc.sync.dma_start(out=bw_sb, in_=bw_d)
            ones = setup.tile([1, 128], F32)
            nc.vector.memset(ones, 1.0)

            hpT = setup.tile([128, KH, 128], F16)
            for k0 in (0, 4):
                pt = sps.tile([128, 512], F16, tag="tp")
                for ki in range(4):
                    k = k0 + ki
                    nc.tensor.transpose(
                        pt[:, ki * 128:(ki + 1) * 128],
                        h_sb[:, k * 128:(k + 1) * 128],
                        ident,
                    )
                nc.scalar.activation(
                    out=hpT[:, k0:k0 + 4, :].rearrange("p a b -> p (a b)"),
                    in_=pt, func=AF.Copy,
                )

            hps = hps_p.tile([128, ATT], F32)
            for k in range(KH):
                wt_sb = setup2.tile([128, ATT], F16, tag="wt")
                nc.scalar.dma_start(
                    out=wt_sb, in_=Wt_d[k * 128:(k + 1) * 128, :])
                for nj in (0, 512):
                    nc.tensor.matmul(
                        hps[:, nj:nj + 512],
                        lhsT=hpT[:, k, :],
                        rhs=wt_sb[:, nj:nj + 512],
                        start=(k == 0), stop=False,
                    )
            for nj in (0, 512):
                nc.tensor.matmul(
                    hps[:, nj:nj + 512],
                    lhsT=ones,
                    rhs=bw_sb[:, nj:nj + 512],
                    start=False, stop=True,
                )
            nc.scalar.activation(out=hattw, in_=hps, func=AF.Copy)

        # ------------- phase A: scores -----------------------------------
        # clamp split: DVE tensor_scalar(max) on [0:P), ACT Relu(-x) on
        # [P:1024).  Fold 1024->128 on PE (identity accumulation), fp32
        # reduce tail on DVE.
        with tc.tile_pool(name="fpw", bufs=3) as fpw_pool, \
                tc.tile_pool(name="zt", bufs=2) as zt_pool, \
                tc.tile_pool(name="fold_ps", bufs=2, space="PSUM") as fold_p:
            for l0, nl in CHUNKS:
                fpw_t = fpw_pool.tile([128, nl * ATT], F16, tag="fpw")
                nc.sync.dma_start(out=fpw_t, in_=fpw_d[:, l0:l0 + nl, :])
                fpw_v = fpw_t.rearrange("p (l a) -> p l a", a=ATT)
                zt = zt_pool.tile([128, nl, ATT], F16, tag="zt")
                hb = bass.AP(
                    tensor=hattw.tensor, offset=hattw.offset,
                    ap=[list(hattw.ap[0]), [0, nl], [1, ATT]],
                )
                # z = fpw + hattw (2x DVE)
                nc.vector.tensor_tensor(
                    out=zt, in0=fpw_v, in1=hb, op=OP.add)
                # clamp in place: [0:P) -> max(.,0) on DVE (4x);
                # [P:1024) -> -relu(-x) ... ACT writes relu(-x); the
                # fold treats those columns via subtract-capable... PE
                # fold can't subtract, so ACT must write min(x,0)
                # directly: min(x,0) = -relu(-x).  ACT has no negate-
                # after-relu, so instead clamp [P:) on DVE too (min)
                # and give ACT the larger pos block when P > HALF.
                nc.vector.tensor_scalar(
                    out=zt[:, :, 0:P], in0=zt[:, :, 0:P],
                    scalar1=0.0, scalar2=None, op0=OP.max)
                nc.vector.tensor_scalar(
                    out=zt[:, :, P:], in0=zt[:, :, P:],
                    scalar1=0.0, scalar2=None, op0=OP.min)
                # PE fold: 8 a-blocks accumulate -> [128, nl*128] psum
                fps = fold_p.tile([128, nl * 128], F32, tag="fold")
                for li in range(nl):
                    for blk in range(8):
                        nc.tensor.matmul(
                            fps[:, li * 128:(li + 1) * 128],
                            lhsT=ident,
                            rhs=zt[:, li, blk * 128:(blk + 1) * 128],
                            start=(blk == 0), stop=(blk == 7),
                        )
                nc.vector.tensor_reduce(
                    out=scores[:, l0:l0 + nl],
                    in_=fps.rearrange("p (l a) -> p l a", a=128),
                    axis=AX.X, op=OP.add,
                )

        # ------------- softmax (no max shift; fp32 exp) ------------------
        sm_s = consts.tile([128, 1], F32)
        sm_r = consts.tile([128, 1], F32)
        e_t = consts.tile([128, L], F32)
        nc.scalar.activation(
            out=e_t, in_=scores, func=AF.Exp, accum_out=sm_s)
        nc.vector.reciprocal(out=sm_r, in_=sm_s)
        nc.vector.tensor_scalar_mul(alpha, e_t, sm_r)

        if dbg:
            nc.gpsimd.dma_start(out=dbg["hattw"], in_=hattw)
            nc.gpsimd.dma_start(out=dbg["scores"], in_=scores)
            nc.gpsimd.dma_start(out=dbg["alpha"], in_=alpha)

        # alpha transposed per step (each aTs tile at partition base 0)
        with tc.tile_pool(name="aps", bufs=2, space="PSUM") as aps:
            for s, (h, off, ns) in enumerate(STEPS):
                l0 = h * L2 + off
                pt = aps.tile([25, 128], F16, tag="apt")
                nc.tensor.transpose(
                    pt[0:ns, :], alpha[:, l0:l0 + ns], ident)
                nc.scalar.activation(
                    out=aTs[s][0:ns, :], in_=pt[0:ns, :], func=AF.Copy)

        # block-diag step matrices (32 small DVE copies, 32-aligned rows)
        Tv = T.rearrange("p (s q j) -> p s q j", q=32, j=4)
        for s, (h, off, ns) in enumerate(STEPS):
            av = aTs[s].rearrange("p (q j) -> p q j", j=4)
            for j in range(4):
                nc.vector.tensor_copy(
                    out=Tv[32 * j:32 * j + ns, s, :, j],
                    in_=av[0:ns, :, j],
                )

        # ------------- phase B: context ----------------------------------
        with tc.tile_pool(name="cps", bufs=4, space="PSUM") as cps_pool:
            z_d = ins["z8k"]
            for q in range(NQ):
                g_t = fb_pool.tile([GP, 8 * ATT], F16, tag="fb")
                if q < 4:
                    # one-time: zero the pad rows [32j+25, 32j+32) of the
                    # 4 rotating buffers (unwritten SBUF may hold NaN bit
                    # patterns; 0 * NaN would poison the matmul psum).
                    for j in range(4):
                        zb = bass.AP(
                            tensor=z_d.tensor, offset=z_d.offset,
                            ap=[[0, 7], [1, 8 * ATT]],
                        )
                        nc.gpsimd.dma_start(
                            out=g_t[32 * j + 25:32 * j + 32, :], in_=zb)
                # G layout [4, 25, NQ, 8, ATT]: each DMA reads 25 rows of
                # 16KB strided NQ*16KB apart -- scattered rows engage HBM
                # bank parallelism (~900 GB/s vs ~300 for contiguous).
                for j in range(4):
                    g_src = bass.AP(
                        tensor=G_d.tensor,
                        offset=G_d.offset + (j * 25 * NQ + q) * 8 * ATT,
                        ap=[[NQ * 8 * ATT, 25], [1, 8 * ATT]],
                    )
                    nc.sync.dma_start(
                        out=g_t[32 * j:32 * j + 25, :], in_=g_src)
                ctxp = cps_pool.tile([4, ATT], F32, tag="ctxp")
                for s, (h, off, ns) in enumerate(STEPS):
                    k = 96 + ns
                    for nj in (0, 512):
                        nc.tensor.matmul(
                            ctxp[:, nj:nj + 512],
                            lhsT=T[0:k, s * 128 + q * 4:s * 128 + q * 4 + 4],
                            rhs=g_t[0:k, s * ATT + nj:s * ATT + nj + 512],
                            start=(s == 0), stop=(s == 7),
                        )
                stage = stage_pool.tile([4, ATT], F32, tag="stage")
                nc.scalar.activation(out=stage, in_=ctxp, func=AF.Copy)
                nc.gpsimd.dma_start(out=ctx_d[4 * q:4 * q + 4, :], in_=stage)


_CACHE = {}


def _build(repeat=1, P=512, debug_outs=False):
    key = (repeat, P, debug_outs)
    if key in _CACHE:
        return _CACHE[key]
    nc = bacc.Bacc(
        "TRN2",
        target_bir_lowering=False,
        debug=False,
        enable_asserts=False,
        num_devices=NCORES,
    )
    ins = {
        "fpw": nc.dram_tensor("fpw", [BS, L, ATT], F16, kind="ExternalInput").ap(),
        "G": nc.dram_tensor("G", [4, 25, NQ, 8, ATT], F16, kind="ExternalInput").ap(),
        "h": nc.dram_tensor("h", [BS, HID], F16, kind="ExternalInput").ap(),
        "Wt": nc.dram_tensor("Wt", [HID, ATT], F16, kind="ExternalInput").ap(),
        "bw": nc.dram_tensor("bw", [ATT], F32, kind="ExternalInput").ap(),
        "z8k": nc.dram_tensor("z8k", [8 * ATT], F16, kind="ExternalInput").ap(),
    }
    outs = {
        "ctx": nc.dram_tensor("ctx", [BS, ATT], F32, kind="ExternalOutput").ap(),
    }
    if debug_outs:
        outs["hattw"] = nc.dram_tensor(
            "hattw", [BS, ATT], F16, kind="ExternalOutput").ap()
        outs["scores"] = nc.dram_tensor(
            "scores", [BS, L], F32, kind="ExternalOutput").ap()
        outs["alpha"] = nc.dram_tensor(
            "alpha", [BS, L], F16, kind="ExternalOutput").ap()
    with tile.TileContext(nc) as tc:
        for _ in range(repeat):
            _emit(tc, outs, ins, P)
    nc.compile()
    _CACHE[key] = nc
    return nc


def _pack_G(f16core):
    """f16core: [BS, L, ATT] fp16 -> G [4, 25, NQ, 8, ATT] (row-scattered)."""
    G = np.zeros((4, 25, NQ, 8, ATT), dtype=np.float16)
    fq = f16core.reshape(NQ, 4, L, ATT)
    for s, (h, off, ns) in enumerate(STEPS):
        l0 = h * L2 + off
        # [NQ, 4, ns, ATT] -> [4, ns, NQ, ATT]
        G[:, 0:ns, :, s, :] = fq[:, :, l0:l0 + ns, :].transpose(1, 2, 0, 3)
    return G


def _host_precondition(features, features_proj, h_prev, W_h, b_h, w_out):
    w = np.asarray(w_out, dtype=np.float32)
    perm = np.argsort(w < 0, kind="stable")  # w>=0 slots first
    P = int((w >= 0).sum())
    Wt = ((w[:, None] * np.asarray(W_h, np.float32))[perm]).T
    Wt = np.ascontiguousarray(Wt, dtype=np.float16)
    bw = (w * np.asarray(b_h, np.float32))[perm].astype(np.float32)
    fpw = (np.asarray(features_proj, np.float32) * w[None, None, :])
    fpw = np.ascontiguousarray(fpw[:, :, perm], dtype=np.float16)
    h16 = np.asarray(h_prev, np.float32).astype(np.float16)
    f16 = np.asarray(features, np.float32).astype(np.float16)
    return fpw, f16, h16, Wt, bw, P


def kernel(features, features_proj, h_prev, W_h, b_h, w_out, b_out=None,
           **kwargs):
    from concourse.bass_utils import run_bass_kernel_spmd

    fpw, f16, h16, Wt, bw, P = _host_precondition(
        features, features_proj, h_prev, W_h, b_h, w_out)

    nc = _build(P=P)
    in_maps = []
    for i in range(NCORES):
        sl = slice(i * BS, (i + 1) * BS)
        in_maps.append({
            "fpw": fpw[sl],
            "G": _pack_G(f16[sl]),
            "h": h16[sl],
            "Wt": Wt,
            "bw": bw,
            "z8k": np.zeros(8 * ATT, np.float16),
        })
    res = run_bass_kernel_spmd(nc, in_maps, core_ids=list(range(NCORES)))
    out = np.concatenate([r["ctx"] for r in res.results], axis=0)
    return out.astype(np.float32)


if __name__ == "__main__":
    rng = np.random.default_rng(0)
    out = kernel(
        features=rng.standard_normal((B, L, ATT), dtype=np.float32),
        features_proj=rng.standard_normal((B, L, ATT), dtype=np.float32),
        h_prev=rng.standard_normal((B, HID), dtype=np.float32),
        W_h=(rng.standard_normal((ATT, HID), dtype=np.float32) * 0.05),
        b_h=(rng.standard_normal((ATT,), dtype=np.float32) * 0.05),
        w_out=(rng.standard_normal((ATT,), dtype=np.float32) * 0.05),
        b_out=np.zeros((1,), dtype=np.float32),
    )
    print(out.shape, out.dtype)


# revision 34
# speedup vs baseline: 4.5908x; 4.5908x over previous
"""Trainium2 Bass kernel for nn_Attention_58815282151556 (sparse_attention).

Reference computation (per batch b):
    h_att  = h_prev @ W_h.T + b_h                       # [B, ATT]
    act    = relu(h_att[:, None, :] + features_proj)    # [B, L, ATT]
    scores = einsum("bla,a->bl", act, w_out) + b_out    # [B, L]
    alpha  = softmax(scores, axis=1)                    # [B, L]
    out    = einsum("bl,bld->bd", alpha, features)      # [B, ATT]

b_out is a constant shift on scores -> softmax-invariant -> dropped exactly.
Sharding: data-parallel over batch, 8 cores x 128 batches, weights
replicated, no cross-core communication.

Host-side preconditioning (exact algebra + fp16 casts + layout only):
  * w is folded into the score path: the kernel streams
    fpw = (features_proj * w)[..., perm] and projects with
    Wt = ((w[:,None]*W_h)[perm]).T, bw = (w*b_h)[perm], where perm sorts
    the attention axis so all w>=0 slots come first (P of them).  With
    zt = w*z = hattw + fpw:
        w_a*relu(z_a) = max(zt_a,0) if w_a>=0 else min(zt_a,0)
    so scores[b,l] is a plain unsigned sum once the clamp direction is
    split at column P.  The permutation only reorders the summed axis.
  * features are host-packed into G[q][p][s][a]: 4-batch groups q, step
    s covering an l-range of 24/25, p = j*ns+li stacking the 4 batches'
    l-slices in the partition dim.  Tiles are [<=100, 16KB rows] -- the
    fast DMA pattern (~900 GB/s/core measured vs ~220 GB/s for
    interleaved 2KB chunks).

Per-core design (103 MB fp16 streams):
  Phase A (scores): fpw streams as [128, 8*1024] fp16 chunks (2 MB,
    16KB rows).  DVE adds hattw broadcast over l (tensor_tensor, 2x),
    the clamp splits between DVE tensor_scalar (max, 4x) on the w>=0
    block and ACT Relu(-x) on the w<0 block, the fold 1024->128 runs on
    the otherwise-idle PE (identity matmuls accumulating 8 a-blocks in
    PSUM), and a DVE tensor_reduce tail emits fp32 scores per l.
  Softmax: no max-subtraction (scores are O(10); fp32 exp is safe);
    exp with accum -> Z, reciprocal, normalize to alpha fp16; PE
    transposes alpha; DVE builds the block-diagonal step matrices.
  Phase B (context): per 4-batch group one [<=100, 8*1024] fp16 tile;
    16 matmuls against block-diag alpha columns accumulate ctx[4,1024]
    in PSUM (K<=100, 8 l-steps x 2 psum halves); 4 PSUM tiles rotate;
    ACT drains [4,1024] tiles (32 ops total) into staging rows DMA'd
    out 4 batches at a time.

Cost-model engine budget: DVE ~160us, ACT ~125us, PE ~195us total
(fold 85 + context 109), DMA ~115us at the measured fast-pattern rate.
"""

import sys

for _p in ("/opt/trn_rl_repo",):
    if _p not in sys.path:
        sys.path.insert(0, _p)

import numpy as np

import concourse.bacc as bacc
import concourse.bass as bass
import concourse.tile as tile
from concourse import mybir
from concourse.masks import make_identity

B, L, ATT, HID = 1024, 196, 1024, 1024
NCORES = 8
BS = B // NCORES  # batches per core
L2 = L // 2  # 98
HALF = ATT // 2  # 512

F32 = mybir.dt.float32
F16 = mybir.dt.float16
OP = mybir.AluOpType
AF = mybir.ActivationFunctionType
AX = mybir.AxisListType

# l-chunking for phase A: 24 chunks of 8 plus one of 4
CHUNKS = [(c * 8, 8) for c in range(24)] + [(192, 4)]

# phase-B l-steps within each half: offsets/sizes covering 98
STEPS_HALF = [(0, 25), (25, 25), (50, 24), (74, 24)]
# (half, l-offset, ns) for all 8 steps
STEPS = [(h, off, ns) for h in (0, 1) for off, ns in STEPS_HALF]
NQ = BS // 4  # 32 four-batch groups
# batch j's rows sit at partition base 32*j (engine partition windows
# must start on 32-aligned bases); rows [32j+ns, 32j+32) are padding.
GP = 128


def _emit(tc, outs, ins, P, phases="AB"):
    nc = tc.nc
    fpw_d = ins["fpw"]  # [BS, L, ATT] fp16
    G_d = ins["G"]  # [NQ, GP, 8, ATT] fp16
    h_d = ins["h"]  # [BS, HID] fp16
    Wt_d = ins["Wt"]  # [HID, ATT] fp16
    bw_d = ins["bw"]  # [ATT] fp32
    ctx_d = outs["ctx"]  # [BS, ATT] fp32

    KH = HID // 128
    dbg = {k: outs[k] for k in ("hattw", "scores", "alpha") if k in outs}

    import contextlib

    with contextlib.ExitStack() as es:
        consts = es.enter_context(tc.tile_pool(name="consts", bufs=1))
        ident = consts.tile([128, 128], F16)
        make_identity(nc, ident)
        hattw = consts.tile([128, ATT], F16)
        scores = consts.tile([128, L], F32)
        alpha = consts.tile([128, L], F16)
        # per-step transposed alpha slices (each at partition base 0)
        aTs = [
            consts.tile([25, 128], F16, name=f"aTs{s}") for s in range(8)
        ]
        # block-diag step matrices: col = s*128 + q*4 + j
        T = consts.tile([GP, 8 * 128], F16)
        nc.gpsimd.memset(T, 0.0)
        fb_pool = es.enter_context(tc.tile_pool(name="fb", bufs=4))
        stage_pool = es.enter_context(tc.tile_pool(name="stg", bufs=2))

        # ------------- setup: hattw = h @ Wt + bw ------------------------
        with tc.tile_pool(name="setup", bufs=1, side="right") as setup, \
                tc.tile_pool(name="setup2", bufs=2, side="right") as setup2, \
                tc.tile_pool(name="setup_ps", bufs=2, space="PSUM") as sps, \
                tc.tile_pool(name="hatt_ps", bufs=1, space="PSUM") as hps_p:
            h_sb = setup.tile([128, HID], F16)
            nc.sync.dma_start(out=h_sb, in_=h_d)
            bw_sb = setup.tile([1, ATT], F32)
            nc.sync.dma_start(out=bw_sb, in_=bw_d)
            ones = setup.tile([1, 128], F32)
            nc.vector.memset(ones, 1.0)

            hpT = setup.tile([128, KH, 128], F16)
            for k0 in (0, 4):
                pt = sps.tile([128, 512], F16, tag="tp")
                for ki in range(4):
                    k = k0 + ki
                    nc.tensor.transpose(
                        pt[:, ki * 128:(ki + 1) * 128],
                        h_sb[:, k * 128:(k + 1) * 128],
                        ident,
                    )
                nc.scalar.activation(
                    out=hpT[:, k0:k0 + 4, :].rearrange("p a b -> p (a b)"),
                    in_=pt, func=AF.Copy,
                )

            hps = hps_p.tile([128, ATT], F32)
            for k in range(KH):
                wt_sb = setup2.tile([128, ATT], F16, tag="wt")
                nc.scalar.dma_start(
                    out=wt_sb, in_=Wt_d[k * 128:(k + 1) * 128, :])
                for nj in (0, 512):
                    nc.tensor.matmul(
                        hps[:, nj:nj + 512],
                        lhsT=hpT[:, k, :],
                        rhs=wt_sb[:, nj:nj + 512],
                        start=(k == 0), stop=False,
                    )
            for nj in (0, 512):
                nc.tensor.matmul(
                    hps[:, nj:nj + 512],
                    lhsT=ones,
                    rhs=bw_sb[:, nj:nj + 512],
                    start=False, stop=True,
                )
            nc.scalar.activation(out=hattw, in_=hps, func=AF.Copy)

        # ------------- phase A: scores -----------------------------------
        # clamp on DVE tensor_scalar (4x): max(.,0) on [0:P), min(.,0) on
        # [P:1024).  Fold 1024->128 on PE (identity accumulation), fp32
        # reduce tail on DVE.
        if "A" not in phases:
            nc.vector.memset(scores, 0.0)
        else:
            with tc.tile_pool(name="fpw", bufs=3) as fpw_pool, \
                    tc.tile_pool(name="zt", bufs=2) as zt_pool, \
                    tc.tile_pool(name="fold_ps", bufs=2, space="PSUM") as fold_p:
                for l0, nl in CHUNKS:
                    fpw_t = fpw_pool.tile([128, nl * ATT], F16, tag="fpw")
                    nc.sync.dma_start(out=fpw_t, in_=fpw_d[:, l0:l0 + nl, :])
                    fpw_v = fpw_t.rearrange("p (l a) -> p l a", a=ATT)
                    zt = zt_pool.tile([128, nl, ATT], F16, tag="zt")
                    hb = bass.AP(
                        tensor=hattw.tensor, offset=hattw.offset,
                        ap=[list(hattw.ap[0]), [0, nl], [1, ATT]],
                    )
                    # z = fpw + hattw (2x DVE)
                    nc.vector.tensor_tensor(
                        out=zt, in0=fpw_v, in1=hb, op=OP.add)
                    # clamp in place (4x DVE)
                    nc.vector.tensor_scalar(
                        out=zt[:, :, 0:P], in0=zt[:, :, 0:P],
                        scalar1=0.0, scalar2=None, op0=OP.max)
                    nc.vector.tensor_scalar(
                        out=zt[:, :, P:], in0=zt[:, :, P:],
                        scalar1=0.0, scalar2=None, op0=OP.min)
                    # PE fold: 8 a-blocks accumulate -> [128, nl*128] psum
                    fps = fold_p.tile([128, nl * 128], F32, tag="fold")
                    for li in range(nl):
                        for blk in range(8):
                            nc.tensor.matmul(
                                fps[:, li * 128:(li + 1) * 128],
                                lhsT=ident,
                                rhs=zt[:, li, blk * 128:(blk + 1) * 128],
                                start=(blk == 0), stop=(blk == 7),
                            )
                    nc.vector.tensor_reduce(
                        out=scores[:, l0:l0 + nl],
                        in_=fps.rearrange("p (l a) -> p l a", a=128),
                        axis=AX.X, op=OP.add,
                    )

        # ------------- softmax (no max shift; fp32 exp) ------------------
        sm_s = consts.tile([128, 1], F32)
        sm_r = consts.tile([128, 1], F32)
        e_t = consts.tile([128, L], F32)
        nc.scalar.activation(
            out=e_t, in_=scores, func=AF.Exp, accum_out=sm_s)
        nc.vector.reciprocal(out=sm_r, in_=sm_s)
        nc.vector.tensor_scalar_mul(alpha, e_t, sm_r)

        if dbg:
            nc.gpsimd.dma_start(out=dbg["hattw"], in_=hattw)
            nc.gpsimd.dma_start(out=dbg["scores"], in_=scores)
            nc.gpsimd.dma_start(out=dbg["alpha"], in_=alpha)

        # alpha transposed per step (each aTs tile at partition base 0)
        with tc.tile_pool(name="aps", bufs=2, space="PSUM") as aps:
            for s, (h, off, ns) in enumerate(STEPS):
                l0 = h * L2 + off
                pt = aps.tile([25, 128], F16, tag="apt")
                nc.tensor.transpose(
                    pt[0:ns, :], alpha[:, l0:l0 + ns], ident)
                nc.scalar.activation(
                    out=aTs[s][0:ns, :], in_=pt[0:ns, :], func=AF.Copy)

        # block-diag step matrices (32 small DVE copies, 32-aligned rows)
        Tv = T.rearrange("p (s q j) -> p s q j", q=32, j=4)
        for s, (h, off, ns) in enumerate(STEPS):
            av = aTs[s].rearrange("p (q j) -> p q j", j=4)
            for j in range(4):
                nc.vector.tensor_copy(
                    out=Tv[32 * j:32 * j + ns, s, :, j],
                    in_=av[0:ns, :, j],
                )

        # ------------- phase B: context ----------------------------------
        if "B" not in phases:
            stage0 = consts.tile([4, ATT], F32)
            nc.vector.memset(stage0, 0.0)
            for q in range(NQ):
                nc.gpsimd.dma_start(
                    out=ctx_d[4 * q:4 * q + 4, :], in_=stage0)
            return
        with tc.tile_pool(name="cps", bufs=4, space="PSUM") as cps_pool:
            for q in range(NQ):
                g_t = fb_pool.tile([GP, 8 * ATT], F16, tag="fb")
                # G layout [128, NQ, 8, ATT] (pad rows are host zeros):
                # one DMA per group, 128 rows of 16KB strided NQ*16KB
                # apart -- scattered rows engage HBM bank parallelism,
                # and one big DMA avoids per-dma_start queue dispatch.
                g_src = bass.AP(
                    tensor=G_d.tensor,
                    offset=G_d.offset + q * 8 * ATT,
                    ap=[[NQ * 8 * ATT, GP], [1, 8 * ATT]],
                )
                nc.sync.dma_start(out=g_t, in_=g_src)
                ctxp = cps_pool.tile([4, ATT], F32, tag="ctxp")
                for s, (h, off, ns) in enumerate(STEPS):
                    k = 96 + ns
                    for nj in (0, 512):
                        nc.tensor.matmul(
                            ctxp[:, nj:nj + 512],
                            lhsT=T[0:k, s * 128 + q * 4:s * 128 + q * 4 + 4],
                            rhs=g_t[0:k, s * ATT + nj:s * ATT + nj + 512],
                            start=(s == 0), stop=(s == 7),
                        )
                stage = stage_pool.tile([4, ATT], F32, tag="stage")
                nc.scalar.activation(out=stage, in_=ctxp, func=AF.Copy)
                nc.gpsimd.dma_start(out=ctx_d[4 * q:4 * q + 4, :], in_=stage)


_CACHE = {}


def _build(repeat=1, P=512, debug_outs=False, phases="AB"):
    key = (repeat, P, debug_outs, phases)
    if key in _CACHE:
        return _CACHE[key]
    nc = bacc.Bacc(
        "TRN2",
        target_bir_lowering=False,
        debug=False,
        enable_asserts=False,
        num_devices=NCORES,
    )
    ins = {
        "fpw": nc.dram_tensor("fpw", [BS, L, ATT], F16, kind="ExternalInput").ap(),
        "G": nc.dram_tensor("G", [GP, NQ, 8, ATT], F16, kind="ExternalInput").ap(),
        "h": nc.dram_tensor("h", [BS, HID], F16, kind="ExternalInput").ap(),
        "Wt": nc.dram_tensor("Wt", [HID, ATT], F16, kind="ExternalInput").ap(),
        "bw": nc.dram_tensor("bw", [ATT], F32, kind="ExternalInput").ap(),
    }
    outs = {
        "ctx": nc.dram_tensor("ctx", [BS, ATT], F32, kind="ExternalOutput").ap(),
    }
    if debug_outs:
        outs["hattw"] = nc.dram_tensor(
            "hattw", [BS, ATT], F16, kind="ExternalOutput").ap()
        outs["scores"] = nc.dram_tensor(
            "scores", [BS, L], F32, kind="ExternalOutput").ap()
        outs["alpha"] = nc.dram_tensor(
            "alpha", [BS, L], F16, kind="ExternalOutput").ap()
    with tile.TileContext(nc) as tc:
        for _ in range(repeat):
            _emit(tc, outs, ins, P, phases=phases)
    nc.compile()
    _CACHE[key] = nc
    return nc


def _pack_G(f16core):
    """f16core: [BS, L, ATT] fp16 -> G [128, NQ, 8, ATT] (row-scattered).

    Row r = 32*j + li holds batch j of the group; li in [25, 32) and
    li >= ns rows stay zero (they multiply zero lhsT columns)."""
    G = np.zeros((GP, NQ, 8, ATT), dtype=np.float16)
    fq = f16core.reshape(NQ, 4, L, ATT)
    for s, (h, off, ns) in enumerate(STEPS):
        l0 = h * L2 + off
        blk = fq[:, :, l0:l0 + ns, :].transpose(1, 2, 0, 3)  # [4,ns,NQ,ATT]
        for j in range(4):
            G[32 * j:32 * j + ns, :, s, :] = blk[j]
    return G


def _host_precondition(features, features_proj, h_prev, W_h, b_h, w_out):
    w = np.asarray(w_out, dtype=np.float32)
    perm = np.argsort(w < 0, kind="stable")  # w>=0 slots first
    P = int((w >= 0).sum())
    Wt = ((w[:, None] * np.asarray(W_h, np.float32))[perm]).T
    Wt = np.ascontiguousarray(Wt, dtype=np.float16)
    bw = (w * np.asarray(b_h, np.float32))[perm].astype(np.float32)
    fpw = (np.asarray(features_proj, np.float32) * w[None, None, :])
    fpw = np.ascontiguousarray(fpw[:, :, perm], dtype=np.float16)
    h16 = np.asarray(h_prev, np.float32).astype(np.float16)
    f16 = np.asarray(features, np.float32).astype(np.float16)
    return fpw, f16, h16, Wt, bw, P


def kernel(features, features_proj, h_prev, W_h, b_h, w_out, b_out=None,
           **kwargs):
    from concourse.bass_utils import run_bass_kernel_spmd

    fpw, f16, h16, Wt, bw, P = _host_precondition(
        features, features_proj, h_prev, W_h, b_h, w_out)

    nc = _build(P=P)
    in_maps = []
    for i in range(NCORES):
        sl = slice(i * BS, (i + 1) * BS)
        in_maps.append({
            "fpw": fpw[sl],
            "G": _pack_G(f16[sl]),
            "h": h16[sl],
            "Wt": Wt,
            "bw": bw,
        })
    res = run_bass_kernel_spmd(nc, in_maps, core_ids=list(range(NCORES)))
    out = np.concatenate([r["ctx"] for r in res.results], axis=0)
    return out.astype(np.float32)


if __name__ == "__main__":
    rng = np.random.default_rng(0)
    out = kernel(
        features=rng.standard_normal((B, L, ATT), dtype=np.float32),
        features_proj=rng.standard_normal((B, L, ATT), dtype=np.float32),
        h_prev=rng.standard_normal((B, HID), dtype=np.float32),
        W_h=(rng.standard_normal((ATT, HID), dtype=np.float32) * 0.05),
        b_h=(rng.standard_normal((ATT,), dtype=np.float32) * 0.05),
        w_out=(rng.standard_normal((ATT,), dtype=np.float32) * 0.05),
        b_out=np.zeros((1,), dtype=np.float32),
    )
    print(out.shape, out.dtype)
